# revision 3
# baseline (speedup 1.0000x reference)
"""Trainium2 Bass kernel for an attention block (B=8, T=2048, D=K=V=1024).

Reference math (per batch element, sharded one per NeuronCore):
    Q = x @ Wq.T + bq ; K = x @ Wk.T + bk ; V = x @ Wv.T + bv
    logits[t,s] = Q[t] . K[s],  masked -inf for s > t (strict upper tri)
    probs = softmax(logits, axis=t) / sqrt(1024)     # softmax over QUERY axis
    out = x + probs @ V

Single fused pipeline, everything resident in SBUF (no DRAM scratch):
  - All matmul operands are bf16 (fp32 PSUM accumulation). Measured rel err
    vs the fp32 reference ~3e-4 (tolerance is 2e-2).
  - Per t-block j: transpose x, project Q/K/V (V does not depend on the
    softmax normalizer, so it lives in the main loop), logits = K^T tiles
    against Q^T block, exp -> P kept in SBUF (lower-triangular blocks only),
    Z accumulated via activation accum_out.
  - softmax over t for fixed s:  Z[s] = sum_t exp(l[t,s]);  read = P' @ Vp
    with Vp[s,:] = V[s,:] / (32 Z[s]).  No max subtraction needed: logits
    are ~N(0,13^2) so exp stays inside fp32/bf16 exponent range.
  - The PV/output phase interleaves into the j=3 logits sweep: PV(i) only
    needs Z[s<=i] which completes right after the j=3 exp for sv=i, so the
    PE runs one continuous instruction stream with no phase boundary.
  - Causal structure skips fully-masked tiles; diagonal 128x512 tiles get an
    additive -1e30 staircase mask before exp.
"""

import time
from contextlib import ExitStack

import numpy as np

import concourse.bass as bass
import concourse.bacc as bacc
import concourse.mybir as mybir
import concourse.tile as tile
from concourse.bass_utils import run_bass_kernel_spmd
from concourse.masks import make_identity

F32 = mybir.dt.float32
F32R = mybir.dt.float32r
BF16 = mybir.dt.bfloat16
AF = mybir.ActivationFunctionType

P = 128          # partitions
T = 2048         # sequence length
D = 1024         # model dim
TB = 512         # t-block width
NTB = T // TB    # 4 t-blocks
DK = D // P      # 8 contraction subtiles
KO = D // P      # 8 k output tiles
SV = T // P      # 16 s strips
HB = D // TB     # 2 v-halves
NEG = -1.0e30
# P-block slot base per j: block (sv, j) lives at slot CUMOFF[j] + sv
CUMOFF = [0, 4, 12, 24]
NSLOT = 40


def _build_nc():
    nc = bacc.Bacc("TRN2", target_bir_lowering=False, debug=False, num_devices=8)

    x = nc.dram_tensor("x", [T, D], F32, kind="ExternalInput").ap()
    Wq = nc.dram_tensor("Wq", [D, D], F32, kind="ExternalInput").ap()
    bq = nc.dram_tensor("bq", [D], F32, kind="ExternalInput").ap()
    Wk = nc.dram_tensor("Wk", [D, D], F32, kind="ExternalInput").ap()
    bk = nc.dram_tensor("bk", [D], F32, kind="ExternalInput").ap()
    Wv = nc.dram_tensor("Wv", [D, D], F32, kind="ExternalInput").ap()
    bv = nc.dram_tensor("bv", [D], F32, kind="ExternalInput").ap()
    out = nc.dram_tensor("out", [T, D], F32, kind="ExternalOutput").ap()

    with tile.TileContext(nc) as tc:
        _kernel_body(nc, tc, x, Wq, bq, Wk, bk, Wv, bv, out)

    nc.compile()
    return nc


def _kernel_body(nc, tc, x, Wq, bq, Wk, bk, Wv, bv, out):
    ctx = ExitStack()
    with ctx:
        consts = ctx.enter_context(tc.tile_pool(name="consts", bufs=1))
        wpool = ctx.enter_context(tc.tile_pool(name="wpool", bufs=3))
        ktpool = ctx.enter_context(tc.tile_pool(name="ktpool", bufs=1))
        ppool = ctx.enter_context(tc.tile_pool(name="ppool", bufs=1))
        vpool = ctx.enter_context(tc.tile_pool(name="vpool", bufs=1))
        qtp = ctx.enter_context(tc.tile_pool(name="qtp", bufs=1))
        psum_mm = ctx.enter_context(tc.tile_pool(name="psum_mm", bufs=5, space="PSUM"))

        # ---- constants ----
        # identity first: it gates every PE transpose at kernel start
        id_f32 = consts.tile([P, P], F32, name="id_f32")
        make_identity(nc, id_f32)
        identity = consts.tile([P, P], F32R, name="identity")
        nc.vector.tensor_copy(out=identity, in_=id_f32)

        # persistent tensors
        KT = ktpool.tile([P, KO, T], BF16, name="KT")          # [k_in, k_out, s]
        Pb = ppool.tile([P, NSLOT, TB], BF16, name="Pb")       # exp(logits) blocks
        Vp = vpool.tile([P, SV, D], BF16, name="Vp")           # V rows, later scaled

        Zacc = consts.tile([P, SV, NTB], F32, name="Zacc")
        zsum = consts.tile([P, SV], F32, name="zsum")
        rtile = consts.tile([P, SV], F32, name="rtile")

        bq_sb = consts.tile([P, KO], F32, name="bq_sb")
        bk_sb = consts.tile([P, KO], F32, name="bk_sb")
        bv_sb = consts.tile([P, D], BF16, name="bv_sb")
        mask_base = consts.tile([P, TB + 3 * P], BF16, name="mask_base")

        def emit_logits(j, sv, qt_blk, masks):
            ps = psum_mm.tile([P, TB], F32, name="ps_l", tag="mm")
            for ko in range(KO):
                nc.tensor.matmul(
                    ps,
                    lhsT=KT[:, ko, sv * P:(sv + 1) * P],
                    rhs=qt_blk[:, ko, :],
                    start=(ko == 0),
                    stop=(ko == KO - 1),
                )
            oi = sv - 4 * j
            if oi >= 0:
                nc.vector.tensor_add(out=ps, in0=ps, in1=masks[oi])
            nc.scalar.activation(
                Pb[:, CUMOFF[j] + sv, :], ps, AF.Exp,
                accum_out=Zacc[:, sv, j:j + 1],
            )

        def emit_qk_proj(j, WT, b_sb, dst_fn, xT_blk):
            for ko in range(KO):
                ps = psum_mm.tile([P, TB], F32, name="ps_p", tag="mm")
                for dk in range(DK):
                    nc.tensor.matmul(
                        ps,
                        lhsT=WT[:, dk, ko * P:(ko + 1) * P],
                        rhs=xT_blk[:, dk, :],
                        start=(dk == 0),
                        stop=(dk == DK - 1),
                    )
                nc.scalar.activation(
                    dst_fn(ko), ps, AF.Identity, bias=b_sb[:, ko:ko + 1]
                )

        def emit_vraw(j, WvT, xT_blk):
            for si in range(TB // P):
                sv = 4 * j + si
                for h in range(HB):
                    ps = psum_mm.tile([P, TB], F32, name="ps_v", tag="mm")
                    for dk in range(DK):
                        nc.tensor.matmul(
                            ps,
                            lhsT=xT_blk[:, dk, si * P:(si + 1) * P],
                            rhs=WvT[:, dk, h * TB:(h + 1) * TB],
                            start=(dk == 0),
                            stop=(dk == DK - 1),
                        )
                    nc.vector.tensor_add(
                        out=Vp[:, sv, h * TB:(h + 1) * TB],
                        in0=ps,
                        in1=bv_sb[:, h * TB:(h + 1) * TB],
                    )

        with (
            tc.tile_pool(name="wnat", bufs=2) as wnat_pool,
            tc.tile_pool(name="xnat", bufs=2) as xnat_pool,
            tc.tile_pool(name="xtp", bufs=2) as xtp,
            tc.tile_pool(name="psum_t", bufs=3, space="PSUM") as psum_t,
        ):
            def make_xT_blk(j, first=False):
                """x t-block -> xT_blk [d_inner, d_outer, t] bf16 via PE."""
                xT_blk = xtp.tile([P, DK, TB], BF16, name="xT_blk", tag="xT")
                for ts_ in range(TB // P):
                    t0 = j * TB + ts_ * P
                    xnat = xnat_pool.tile([P, D], F32R, name="xnat", tag="xnat")
                    eng = nc.sync if first else nc.gpsimd
                    for q in range(2):
                        h0 = q * (P // 2)
                        eng.dma_start(
                            out=xnat[h0:h0 + P // 2, :],
                            in_=x[t0 + h0:t0 + h0 + P // 2, :].bitcast(F32R))
                    for g in range(2):
                        ptg = psum_t.tile([P, 4 * P], F32R, name="ptg", tag="pt")
                        for q in range(4):
                            dk = g * 4 + q
                            nc.tensor.transpose(
                                ptg[:, q * P:(q + 1) * P],
                                xnat[:, dk * P:(dk + 1) * P],
                                identity,
                            )
                        nc.vector.tensor_copy(
                            out=xT_blk[:, g * 4:(g + 1) * 4, ts_ * P:(ts_ + 1) * P],
                            in_=ptg.rearrange("p (a b) -> p a b", a=4),
                        )
                return xT_blk

            def transpose_weight(w_ap, dst):
                """[1024,1024] DRAM weight -> dst [d_in, d_out, k] bf16."""
                for kt in range(8):
                    wnat = wnat_pool.tile([P, D], F32R, name="wnat", tag="wnat")
                    eng_a = nc.gpsimd if kt % 2 == 0 else nc.sync
                    eng_b = nc.sync if kt % 2 == 0 else nc.gpsimd
                    eng_a.dma_start(out=wnat[:P // 2, :],
                                    in_=w_ap[kt * P:kt * P + P // 2, :].bitcast(F32R))
                    eng_b.dma_start(out=wnat[P // 2:, :],
                                    in_=w_ap[kt * P + P // 2:(kt + 1) * P, :].bitcast(F32R))
                    for g in range(2):
                        ptg = psum_t.tile([P, 4 * P], F32R, name="ptg", tag="pt")
                        for q in range(4):
                            dk = g * 4 + q
                            nc.tensor.transpose(
                                ptg[:, q * P:(q + 1) * P],
                                wnat[:, dk * P:(dk + 1) * P],
                                identity,
                            )
                        ce = nc.vector if (kt * 2 + g) % 2 == 0 else nc.scalar
                        src = ptg.rearrange("p (a b) -> p a b", a=4)
                        dst_ap = dst[:, g * 4:(g + 1) * 4, kt * P:(kt + 1) * P]
                        if ce is nc.vector:
                            ce.tensor_copy(out=dst_ap, in_=src)
                        else:
                            ce.copy(out=dst_ap, in_=src)

            # j=0 x tiles on the fast HWDGE path; they gate the first PE work
            xT0 = make_xT_blk(0, first=True)

            # tiny bias loads ride the sync queue right behind the x tiles
            nc.sync.dma_start(out=bq_sb, in_=bq.rearrange("(o p) -> p o", p=P))
            nc.sync.dma_start(out=bk_sb, in_=bk.rearrange("(o p) -> p o", p=P))

            # weight transposes interleaved with j=0 projections so the PE
            # works on Q-proj/K-proj while Wk/Wv/x1 DMAs stream in; queue
            # emission order tracks need-by order: x0, Wq, Wk, x1, Wv, ...
            WqT = wpool.tile([P, DK, D], BF16, name="WqT", tag="W")
            transpose_weight(Wq, WqT)
            WkT = wpool.tile([P, DK, D], BF16, name="WkT", tag="W")
            WvT = wpool.tile([P, DK, D], BF16, name="WvT", tag="W")

            xT_blks = [xT0, None, None, None]
            qt_blk = None
            for j in range(NTB):
                xT_blk = xT_blks[j]
                qt_blk = qtp.tile([P, KO, TB], BF16, name="qt_blk", tag="qt")
                emit_qk_proj(j, WqT, bq_sb,
                             lambda ko: qt_blk[:, ko, :], xT_blk)
                if j == 0:
                    transpose_weight(Wk, WkT)
                    xT_blks[1] = make_xT_blk(1)
                emit_qk_proj(j, WkT, bk_sb,
                             lambda ko: KT[:, ko, j * TB:(j + 1) * TB], xT_blk)
                if j == 0:
                    transpose_weight(Wv, WvT)
                    # constants needed from Vraw(0)/logits(0) on: emitted
                    # behind the Wv DMA halves in the gpsimd queue
                    bv_bcast = bass.AP(tensor=bv.tensor, offset=bv.offset,
                                       ap=[[0, P], [1, D]])
                    nc.gpsimd.dma_start(out=bv_sb, in_=bv_bcast)
                    nc.vector.memset(Zacc, 0.0)
                    # sliding staircase mask [128, 896]: valid (0.0) iff
                    # f >= p + 384, else -1e30; the mask for diagonal offset
                    # oi*128 is a 512-wide slice.
                    nc.gpsimd.memset(mask_base, 0.0)
                    nc.gpsimd.affine_select(
                        out=mask_base, in_=mask_base,
                        compare_op=mybir.AluOpType.is_ge,
                        fill=NEG,
                        base=-(3 * P),
                        pattern=[[1, TB + 3 * P]],
                        channel_multiplier=-1,
                    )
                masks = [mask_base[:, 3 * P - oi * P: 3 * P - oi * P + TB]
                         for oi in range(4)]
                emit_vraw(j, WvT, xT_blk)
                if j == NTB - 1:
                    break  # j=3 logits are fused with the PV phase below
                for sv in range(4 * (j + 1)):
                    emit_logits(j, sv, qt_blk, masks)
                    # mid-sweep: queue the next-next x block's transposes so
                    # their DMAs have a full logits sweep to land
                    if sv == 1 and j + 2 < NTB:
                        xT_blks[j + 2] = make_xT_blk(j + 2)

        # ---- fused j=3 logits + PV/output phase ----
        # PV(i) needs rtile[sv<=i], complete right after the j=3 exp of sv=i.
        with (
            tc.tile_pool(name="xres", bufs=3) as xres_pool,
            tc.tile_pool(name="ost", bufs=2) as ost_pool,
        ):
            xres_tiles = {}

            def prefetch_xres(i):
                xr = xres_pool.tile([P, D], F32, name="xres", tag="xres")
                nc.gpsimd.dma_start(out=xr, in_=x[i * P:(i + 1) * P, :])
                xres_tiles[i] = xr

            def emit_tailpiece(i):
                # rtile[i] = 1 / (32 * sum_j Zacc[:, i, j])
                nc.vector.reduce_sum(out=zsum[:, i:i + 1], in_=Zacc[:, i, :],
                                     axis=mybir.AxisListType.X)
                nc.vector.reciprocal(rtile[:, i:i + 1], zsum[:, i:i + 1])
                nc.vector.tensor_scalar_mul(rtile[:, i:i + 1],
                                            rtile[:, i:i + 1], 1.0 / 32.0)
                nc.vector.tensor_scalar_mul(
                    Vp[:, i, :], Vp[:, i, :], rtile[:, i:i + 1])
                if i + 2 < SV:
                    prefetch_xres(i + 2)
                jj = i // 4
                toff = (i % 4) * P
                ost = ost_pool.tile([P, D], F32, name="ost", tag="ost")
                for h in range(HB):
                    ps = psum_mm.tile([P, TB], F32, name="ps_o", tag="mm")
                    for svv in range(i + 1):
                        nc.tensor.matmul(
                            ps,
                            lhsT=Pb[:, CUMOFF[jj] + svv, toff:toff + P],
                            rhs=Vp[:, svv, h * TB:(h + 1) * TB],
                            start=(svv == 0),
                            stop=(svv == i),
                        )
                    nc.vector.tensor_add(
                        out=ost[:, h * TB:(h + 1) * TB],
                        in0=ps,
                        in1=xres_tiles[i][:, h * TB:(h + 1) * TB],
                    )
                    nc.sync.dma_start(
                        out=out[i * P:(i + 1) * P, h * TB:(h + 1) * TB],
                        in_=ost[:, h * TB:(h + 1) * TB],
                    )

            prefetch_xres(0)
            prefetch_xres(1)
            for sv in range(SV):
                emit_logits(3, sv, qt_blk, masks)
                if sv >= 1:
                    emit_tailpiece(sv - 1)
            emit_tailpiece(SV - 1)


_NC_CACHE = None


def _get_nc():
    global _NC_CACHE
    if _NC_CACHE is None:
        _NC_CACHE = _build_nc()
    return _NC_CACHE


def kernel(minibatch, Wq, bq, Wk, bk, Wv, bv):
    minibatch = np.asarray(minibatch, dtype=np.float32)
    Wq = np.asarray(Wq, dtype=np.float32)
    bq = np.asarray(bq, dtype=np.float32)
    Wk = np.asarray(Wk, dtype=np.float32)
    bk = np.asarray(bk, dtype=np.float32)
    Wv = np.asarray(Wv, dtype=np.float32)
    bv = np.asarray(bv, dtype=np.float32)

    nc = _get_nc()
    B = minibatch.shape[0]
    in_maps = [
        {
            "x": np.ascontiguousarray(minibatch[i]),
            "Wq": Wq, "bq": bq, "Wk": Wk, "bk": bk, "Wv": Wv, "bv": bv,
        }
        for i in range(B)
    ]
    last_err = None
    for _attempt in range(3):
        try:
            res = run_bass_kernel_spmd(nc, in_maps, core_ids=list(range(B)))
            break
        except Exception as e:  # transient device errors (e.g. NRT_EXEC_UNIT_UNRECOVERABLE)
            last_err = e
            time.sleep(2.0)
    else:
        raise last_err
    return np.stack([res.results[i]["out"] for i in range(B)], axis=0)


# revision 33
# speedup vs baseline: 1.6540x; 1.6540x over previous
"""Trainium2 Bass kernel for an attention block (B=8, T=2048, D=K=V=1024).

Reference math (per batch element, sharded one per NeuronCore):
    Q = x @ Wq.T + bq ; K = x @ Wk.T + bk ; V = x @ Wv.T + bv
    logits[t,s] = Q[t] . K[s],  masked -inf for s > t (strict upper tri)
    probs = softmax(logits, axis=t) / sqrt(1024)     # softmax over QUERY axis
    out = x + probs @ V

Single fused pipeline, everything resident in SBUF (no DRAM scratch):
  - All matmul operands are bf16 (fp32 PSUM accumulation). Measured rel err
    vs the fp32 reference ~3e-4 (tolerance is 2e-2).
  - Per t-block j: transpose x, project Q/K/V (V does not depend on the
    softmax normalizer, so it lives in the main loop), logits = K^T tiles
    against Q^T block, exp -> P kept in SBUF (lower-triangular blocks only),
    Z accumulated via activation accum_out.
  - softmax over t for fixed s:  Z[s] = sum_t exp(l[t,s]);  read = P' @ Vp
    with Vp[s,:] = V[s,:] / (32 Z[s]).  No max subtraction needed: logits
    are ~N(0,13^2) so exp stays inside fp32/bf16 exponent range.
  - The PV/output phase interleaves into the j=3 logits sweep: PV(i) only
    needs Z[s<=i] which completes right after the j=3 exp for sv=i, so the
    PE runs one continuous instruction stream with no phase boundary.
  - Causal structure skips fully-masked tiles; diagonal 128x512 tiles get an
    additive -1e30 staircase mask before exp.
"""

import time
from contextlib import ExitStack

import numpy as np

import concourse.bass as bass
import concourse.bacc as bacc
import concourse.mybir as mybir
import concourse.tile as tile
from concourse.bass_utils import run_bass_kernel_spmd
from concourse.masks import make_identity

F32 = mybir.dt.float32
F32R = mybir.dt.float32r
BF16 = mybir.dt.bfloat16
F8 = mybir.dt.float8e4
AF = mybir.ActivationFunctionType
WSCALE = 64.0  # Wv pre-scale so fp8e4m3 covers the 0.02-std weight range

P = 128          # partitions
T = 2048         # sequence length
D = 1024         # model dim
TB = 512         # t-block width
NTB = T // TB    # 4 t-blocks
DK = D // P      # 8 contraction subtiles
KO = D // P      # 8 k output tiles
SV = T // P      # 16 s strips
HB = D // TB     # 2 v-halves
NEG = -1.0e30
# P-block slot base per j: block (sv, j) lives at slot CUMOFF[j] + sv
CUMOFF = [0, 4, 12, 24]
NSLOT = 40


def _build_nc():
    nc = bacc.Bacc("TRN2", target_bir_lowering=False, debug=False, num_devices=8)

    x = nc.dram_tensor("x", [T, D], F32, kind="ExternalInput").ap()
    Wq = nc.dram_tensor("Wq", [D, D], F32, kind="ExternalInput").ap()
    bq = nc.dram_tensor("bq", [D], F32, kind="ExternalInput").ap()
    Wk = nc.dram_tensor("Wk", [D, D], F32, kind="ExternalInput").ap()
    bk = nc.dram_tensor("bk", [D], F32, kind="ExternalInput").ap()
    Wv = nc.dram_tensor("Wv", [D, D], F32, kind="ExternalInput").ap()
    bv = nc.dram_tensor("bv", [D], F32, kind="ExternalInput").ap()
    out = nc.dram_tensor("out", [T, D], F32, kind="ExternalOutput").ap()

    with tile.TileContext(nc) as tc:
        _kernel_body(nc, tc, x, Wq, bq, Wk, bk, Wv, bv, out)

    nc.compile()
    return nc


def _kernel_body(nc, tc, x, Wq, bq, Wk, bk, Wv, bv, out):
    ctx = ExitStack()
    with ctx:
        consts = ctx.enter_context(tc.tile_pool(name="consts", bufs=1))
        wpool = ctx.enter_context(tc.tile_pool(name="wpool", bufs=2))
        w8pool = ctx.enter_context(tc.tile_pool(name="w8pool", bufs=1))
        ktpool = ctx.enter_context(tc.tile_pool(name="ktpool", bufs=1))
        ppool = ctx.enter_context(tc.tile_pool(name="ppool", bufs=1))
        vpool = ctx.enter_context(tc.tile_pool(name="vpool", bufs=1))
        qtp = ctx.enter_context(tc.tile_pool(name="qtp", bufs=1))
        psum_mm = ctx.enter_context(tc.tile_pool(name="psum_mm", bufs=5, space="PSUM"))

        # ---- constants ----
        # identity first: it gates every PE transpose at kernel start.
        # bf16 throughout: staging tiles are cast to bf16 during the DMA
        # (SWDGE), and bf16 PE transposes run at 1.0 cycles/row vs 1.5 f32r.
        identity = consts.tile([P, P], BF16, name="identity")
        make_identity(nc, identity)

        # persistent tensors
        KT = ktpool.tile([P, KO, T], BF16, name="KT")          # [k_in, k_out, s]
        Pb = ppool.tile([P, NSLOT, TB], BF16, name="Pb")       # exp(logits) blocks
        Vp = vpool.tile([P, SV, D], F8, name="Vp")             # V rows, fp8

        # Z accumulator: slots [0..3] hold full-block sums per j; slots
        # [4..7] hold the diagonal 128-wide sub-block sums (piece c).
        NZ = NTB + 4
        Zacc = consts.tile([P, SV, NZ], F32, name="Zacc")
        zsum = consts.tile([P, SV], F32, name="zsum")
        rtile = consts.tile([P, SV], F32, name="rtile")

        bq_sb = consts.tile([P, KO], F32, name="bq_sb")
        bk_sb = consts.tile([P, KO], F32, name="bk_sb")
        bv_sb = consts.tile([P, D], BF16, name="bv_sb")
        # [128,128] causal mask for the diagonal sub-block: 0.0 iff f >= p
        dmask = consts.tile([P, P], BF16, name="dmask")

        def emit_logits(j, sv, qt_blk):
            oi = sv - 4 * j
            if oi < 0:
                # fully-unmasked [128 s, 512 t] block
                ps = psum_mm.tile([P, TB], F32, name="ps_l", tag="mm")
                for ko in range(KO):
                    nc.tensor.matmul(
                        ps,
                        lhsT=KT[:, ko, sv * P:(sv + 1) * P],
                        rhs=qt_blk[:, ko, :],
                        start=(ko == 0),
                        stop=(ko == KO - 1),
                    )
                nc.scalar.activation(
                    Pb[:, CUMOFF[j] + sv, :], ps, AF.Exp,
                    accum_out=Zacc[:, sv, j:j + 1],
                )
                return
            # diagonal: only the t >= s sub-blocks (c >= oi), 128 wide; all
            # pieces accumulate into disjoint column ranges of ONE psum bank
            psf = psum_mm.tile([P, TB], F32, name="ps_l", tag="mm")
            for c in range(oi, 4):
                ps = psf[:, c * P:(c + 1) * P]
                for ko in range(KO):
                    nc.tensor.matmul(
                        ps,
                        lhsT=KT[:, ko, sv * P:(sv + 1) * P],
                        rhs=qt_blk[:, ko, c * P:(c + 1) * P],
                        start=(ko == 0),
                        stop=(ko == KO - 1),
                    )
                if c == oi:
                    nc.vector.tensor_add(out=ps, in0=ps, in1=dmask)
                nc.scalar.activation(
                    Pb[:, CUMOFF[j] + sv, c * P:(c + 1) * P], ps, AF.Exp,
                    accum_out=Zacc[:, sv, 4 + c:5 + c],
                )

        def emit_qk_proj(j, WT, b_sb, dst_fn, xT_blk):
            for ko in range(KO):
                ps = psum_mm.tile([P, TB], F32, name="ps_p", tag="mm")
                for dk in range(DK):
                    nc.tensor.matmul(
                        ps,
                        lhsT=WT[:, dk, ko * P:(ko + 1) * P],
                        rhs=xT_blk[:, dk, :],
                        start=(dk == 0),
                        stop=(dk == DK - 1),
                    )
                nc.scalar.activation(
                    dst_fn(ko), ps, AF.Identity, bias=b_sb[:, ko:ko + 1]
                )

        def emit_vraw(j, WvT8, xT8_blk):
            # fp8e4m3 DoubleRow: 2 contraction sub-tiles per matmul, 2
            # multiplies/cycle.  Wv was scaled by WSCALE into fp8 range;
            # undone in the epilogue (out = ps/WSCALE + bv).
            for si in range(TB // P):
                sv = 4 * j + si
                for h in range(HB):
                    ps = psum_mm.tile([P, TB], F32, name="ps_v", tag="mm")
                    for p4 in range(DK // 2):
                        nc.tensor.matmul(
                            ps,
                            lhsT=xT8_blk[:, 2 * p4:2 * p4 + 2, si * P:(si + 1) * P],
                            rhs=WvT8[:, 2 * p4:2 * p4 + 2, h * TB:(h + 1) * TB],
                            start=(p4 == 0),
                            stop=(p4 == DK // 2 - 1),
                            perf_mode=mybir.MatmulPerfMode.DoubleRow,
                        )
                    nc.vector.scalar_tensor_tensor(
                        out=Vp[:, sv, h * TB:(h + 1) * TB],
                        in0=ps,
                        scalar=1.0 / WSCALE,
                        in1=bv_sb[:, h * TB:(h + 1) * TB],
                        op0=mybir.AluOpType.mult,
                        op1=mybir.AluOpType.add,
                    )

        with (
            tc.tile_pool(name="nat", bufs=12) as nat_pool,
            tc.tile_pool(name="xtp", bufs=2) as xtp,
            tc.tile_pool(name="xtp8", bufs=2) as xtp8,
            tc.tile_pool(name="psum_t", bufs=3, space="PSUM") as psum_t,
        ):
            # staging tiles are bf16, filled by SWDGE cast-DMA (f32 DRAM ->
            # bf16 SBUF); a deep shared pool lets the whole DMA stream run
            # ahead of the PE transposes
            def stage_tile(src_rows):
                nat = nat_pool.tile([P, D], BF16, name="nat", tag="nat")
                nc.gpsimd.dma_start(out=nat, in_=src_rows)
                return nat

            def transpose_in(nat, dst_fns):
                """8 PE transposes of one [128,1024] tile + grouped copies.

                dst_fns: per-group list of (engine, dst_ap, scale) copy sinks.
                """
                for g in range(2):
                    ptg = psum_t.tile([P, 4 * P], BF16, name="ptg", tag="pt")
                    for q in range(4):
                        dk = g * 4 + q
                        nc.tensor.transpose(
                            ptg[:, q * P:(q + 1) * P],
                            nat[:, dk * P:(dk + 1) * P],
                            identity,
                        )
                    src = ptg.rearrange("p (a b) -> p a b", a=4)
                    for ce, dst_ap, scale in dst_fns(g):
                        if scale is not None:
                            if ce is nc.vector:
                                ce.tensor_scalar_mul(dst_ap, src, scale)
                            else:
                                ce.activation(dst_ap, src, AF.Identity,
                                              scale=scale)
                        elif ce is nc.vector:
                            ce.tensor_copy(out=dst_ap, in_=src)
                        else:
                            ce.copy(out=dst_ap, in_=src)

            def make_xT_blk(j):
                """x t-block -> xT [d_in, d_out, t] bf16 + fp8 via PE."""
                xT_blk = xtp.tile([P, DK, TB], BF16, name="xT_blk", tag="xT")
                xT8_blk = xtp8.tile([P, DK, TB], F8, name="xT8_blk", tag="xT8")
                for ts_ in range(TB // P):
                    t0 = j * TB + ts_ * P
                    nat = stage_tile(x[t0:t0 + P, :])
                    transpose_in(nat, lambda g: [
                        (nc.vector,
                         xT_blk[:, g * 4:(g + 1) * 4, ts_ * P:(ts_ + 1) * P],
                         None),
                        (nc.scalar,
                         xT8_blk[:, g * 4:(g + 1) * 4, ts_ * P:(ts_ + 1) * P],
                         None),
                    ])
                return xT_blk, xT8_blk

            def transpose_weight(w_ap, dst, scale=None):
                """[1024,1024] DRAM weight -> dst [d_in, d_out, k]."""
                for kt in range(8):
                    nat = stage_tile(w_ap[kt * P:(kt + 1) * P, :])
                    transpose_in(nat, lambda g, kt=kt: [
                        (nc.vector if (kt * 2 + g) % 2 == 0 else nc.scalar,
                         dst[:, g * 4:(g + 1) * 4, kt * P:(kt + 1) * P],
                         scale)])

            # j=0 x tiles first: they gate the first PE work
            xT0 = make_xT_blk(0)

            # tiny bias loads ride the sync queue right behind the x tiles
            nc.sync.dma_start(out=bq_sb, in_=bq.rearrange("(o p) -> p o", p=P))
            nc.sync.dma_start(out=bk_sb, in_=bk.rearrange("(o p) -> p o", p=P))

            # weight transposes interleaved with j=0 projections so the PE
            # works on Q-proj/K-proj while Wk/Wv/x1 DMAs stream in; queue
            # emission order tracks need-by order: x0, Wq, Wk, x1, Wv, ...
            WqT = wpool.tile([P, DK, D], BF16, name="WqT", tag="W")
            transpose_weight(Wq, WqT)
            WkT = wpool.tile([P, DK, D], BF16, name="WkT", tag="W")
            WvT8 = w8pool.tile([P, DK, D], F8, name="WvT8", tag="W8")

            xT_blks = [xT0, None, None, None]
            qt_blk = None
            for j in range(NTB):
                xT_blk, xT8_blk = xT_blks[j]
                qt_blk = qtp.tile([P, KO, TB], BF16, name="qt_blk", tag="qt")
                emit_qk_proj(j, WqT, bq_sb,
                             lambda ko: qt_blk[:, ko, :], xT_blk)
                if j == 0:
                    transpose_weight(Wk, WkT)
                    xT_blks[1] = make_xT_blk(1)
                emit_qk_proj(j, WkT, bk_sb,
                             lambda ko: KT[:, ko, j * TB:(j + 1) * TB], xT_blk)
                if j == 0:
                    transpose_weight(Wv, WvT8, scale=WSCALE)
                    # constants needed from Vraw(0)/logits(0) on: emitted
                    # behind the Wv DMAs in the gpsimd queue
                    bv_bcast = bass.AP(tensor=bv.tensor, offset=bv.offset,
                                       ap=[[0, P], [1, D]])
                    nc.gpsimd.dma_start(out=bv_sb, in_=bv_bcast)
                    nc.vector.memset(Zacc, 0.0)
                    # causal mask for diagonal sub-blocks: 0.0 iff f >= p
                    nc.gpsimd.memset(dmask, 0.0)
                    nc.gpsimd.affine_select(
                        out=dmask, in_=dmask,
                        compare_op=mybir.AluOpType.is_ge,
                        fill=NEG,
                        base=0,
                        pattern=[[1, P]],
                        channel_multiplier=-1,
                    )
                emit_vraw(j, WvT8, xT8_blk)
                if j == NTB - 1:
                    break  # j=3 logits are fused with the PV phase below
                for sv in range(4 * (j + 1)):
                    emit_logits(j, sv, qt_blk)
                    # mid-sweep: queue the next-next x block's transposes so
                    # their DMAs have a full logits sweep to land
                    if sv == 1 and j + 2 < NTB:
                        xT_blks[j + 2] = make_xT_blk(j + 2)

        # ---- fused j=3 logits + PV/output phase ----
        # PV(i) needs 1/Z[sv<=i], complete right after the j=3 exp of sv=i.
        # P blocks are rescaled by 1/Z (range [0,1]) and cast to fp8 so the
        # PV contraction runs as fp8 DoubleRow; the softmax 1/32 is folded
        # into the output epilogue.
        with (
            tc.tile_pool(name="p8pool", bufs=1) as p8pool,
            tc.tile_pool(name="xres", bufs=3) as xres_pool,
            tc.tile_pool(name="ost", bufs=2) as ost_pool,
        ):
            P8 = p8pool.tile([P, NSLOT, TB], F8, name="P8")
            xres_tiles = {}

            def prefetch_xres(i):
                xr = xres_pool.tile([P, D], F32, name="xres", tag="xres")
                nc.sync.dma_start(out=xr, in_=x[i * P:(i + 1) * P, :])
                xres_tiles[i] = xr

            def emit_tailpiece(i):
                # rtile[i] = 1 / Z[i]
                nc.vector.reduce_sum(out=zsum[:, i:i + 1], in_=Zacc[:, i, :],
                                     axis=mybir.AxisListType.X)
                nc.vector.reciprocal(rtile[:, i:i + 1], zsum[:, i:i + 1])
                # normalize+cast every P block of strip i to fp8
                for j in range(i // 4, NTB):
                    sl = CUMOFF[j] + i
                    if (i + j) % 2 == 0:
                        nc.vector.tensor_scalar_mul(
                            P8[:, sl, :], Pb[:, sl, :], rtile[:, i:i + 1])
                    else:
                        nc.scalar.activation(
                            P8[:, sl, :], Pb[:, sl, :], AF.Identity,
                            scale=rtile[:, i:i + 1])
                if i + 2 < SV:
                    prefetch_xres(i + 2)
                jj = i // 4
                toff = (i % 4) * P
                npair = (i + 1) // 2
                odd = (i + 1) % 2
                ost = ost_pool.tile([P, D], F32, name="ost", tag="ost")
                for h in range(HB):
                    ps = psum_mm.tile([P, TB], F32, name="ps_o", tag="mm")
                    for pp in range(npair):
                        nc.tensor.matmul(
                            ps,
                            lhsT=P8[:, CUMOFF[jj] + 2 * pp:CUMOFF[jj] + 2 * pp + 2,
                                    toff:toff + P],
                            rhs=Vp[:, 2 * pp:2 * pp + 2, h * TB:(h + 1) * TB],
                            start=(pp == 0),
                            stop=(pp == npair - 1 and not odd),
                            perf_mode=mybir.MatmulPerfMode.DoubleRow,
                        )
                    if odd:
                        nc.tensor.matmul(
                            ps,
                            lhsT=P8[:, CUMOFF[jj] + i, toff:toff + P],
                            rhs=Vp[:, i, h * TB:(h + 1) * TB],
                            start=(npair == 0),
                            stop=True,
                        )
                    nc.vector.scalar_tensor_tensor(
                        out=ost[:, h * TB:(h + 1) * TB],
                        in0=ps,
                        scalar=1.0 / 32.0,
                        in1=xres_tiles[i][:, h * TB:(h + 1) * TB],
                        op0=mybir.AluOpType.mult,
                        op1=mybir.AluOpType.add,
                    )
                    nc.sync.dma_start(
                        out=out[i * P:(i + 1) * P, h * TB:(h + 1) * TB],
                        in_=ost[:, h * TB:(h + 1) * TB],
                    )

            prefetch_xres(0)
            prefetch_xres(1)
            for sv in range(SV):
                emit_logits(3, sv, qt_blk)
                if sv >= 1:
                    emit_tailpiece(sv - 1)
            emit_tailpiece(SV - 1)


_NC_CACHE = None


def _get_nc():
    global _NC_CACHE
    if _NC_CACHE is None:
        _NC_CACHE = _build_nc()
    return _NC_CACHE


def kernel(minibatch, Wq, bq, Wk, bk, Wv, bv):
    minibatch = np.asarray(minibatch, dtype=np.float32)
    Wq = np.asarray(Wq, dtype=np.float32)
    bq = np.asarray(bq, dtype=np.float32)
    Wk = np.asarray(Wk, dtype=np.float32)
    bk = np.asarray(bk, dtype=np.float32)
    Wv = np.asarray(Wv, dtype=np.float32)
    bv = np.asarray(bv, dtype=np.float32)

    nc = _get_nc()
    B = minibatch.shape[0]
    in_maps = [
        {
            "x": np.ascontiguousarray(minibatch[i]),
            "Wq": Wq, "bq": bq, "Wk": Wk, "bk": bk, "Wv": Wv, "bv": bv,
        }
        for i in range(B)
    ]
    last_err = None
    for _attempt in range(3):
        try:
            res = run_bass_kernel_spmd(nc, in_maps, core_ids=list(range(B)))
            break
        except Exception as e:  # transient device errors (e.g. NRT_EXEC_UNIT_UNRECOVERABLE)
            last_err = e
            time.sleep(2.0)
    else:
        raise last_err
    return np.stack([res.results[i]["out"] for i in range(B)], axis=0)
